# revision 1
# baseline (speedup 1.0000x reference)
"""Trainium2 Bass kernel for nn_BlockSelfAttentionModule.

Reference semantics (B=4, H=8, L=1024, I=16 instruments, F=64 frames, D=64):
  out[b*H+h, l, m] = q[l] . r_instrument[l%I, m%I, :, h]
                   + q[l] . a_h[(l//I - m//I) mod (F+1)]
  where a_h = concat(e_past[:, :, h], -111 pad row)   # (65, D)

Both bias terms factor through small per-row tables:
  Ui[l, c]  = q[l] . R_h[l%I, c]           (L x 16)
  Psh[l, f] = q[l] . a_h[(l//I - f) % 65]  (L x 64)
  out[l, f*16 + c] = Psh[l, f] + Ui[l, c]

Strategy (8 cores data-parallel over the 32 = B*H rows, 4 rows/core;
all-bf16, rel-err ~5e-3 against the 2e-2 gate):
  * bf16 output halves the dominant HBM write (16.8 -> 8.4 MiB/core).
    DMA transfers serialize on the ISSUING engine (~332 GB/s each, cost
    ~ free-bytes, partition-count-free), so per-tile output DMAs are
    spread across SP, Activation, and Pool(SWDGE) for ~1 TB/s aggregate.
  * Host pre-builds per (b*H+h) row: the block-diagonal zero-padded zq
    (so K=128 matmuls cover 2 frames at once), a packed qT|rt table, and
    the shared reversed-diagonal time table a2.
  * PE: 16 ui matmuls -> Ui^T in one batched PSUM tile; 16 psh matmuls
    per half-row into one batched PSUM tile; 8 transposes of Ui^T into
    one PSUM tile. All matmul operands start at partition 0 (operands at
    partition offset 64 crash the exec unit) and GPSIMD never touches
    PSUM (illegal) — PSUM evacuation is batched on DVE: one copy per
    half-row doubles Psh into pairs (p2sb[l, 2f+c2]) and one copy per
    row moves the transposed Ui, minimizing per-op PSUM access cost.
  * The full-size expansion out[l, (f, c-hi, c2)] = psh2 + ui runs as
    tensor_tensor adds whose operands are all 2-byte packed (last dim
    count 2), hitting DVE's 2x mode (594 ns/tile); a tuned share of
    tiles runs on the otherwise-idle Pool engine (853 ns/tile), with
    per-tile DMA engine assignment balancing all five engines.
Host casts the returned bf16 (4, L, L) blocks back to f32.
"""

import numpy as np
import ml_dtypes

import concourse.bass as bass
import concourse.bacc as bacc
import concourse.mybir as mybir
from concourse import masks
from concourse.tile import TileContext
from concourse.bass_utils import run_bass_kernel_spmd

F32 = mybir.dt.float32
BF16 = mybir.dt.bfloat16
NP_BF16 = ml_dtypes.bfloat16

N_CORES = 8
ROWS_PER_CORE = 4  # (b*H + h) rows per core
L = 1024
D = 64
I = 16
F = 64
PAD_VAL = -111.0

IT_COLS = 1024 + 256  # qT | rt

# Per-tile assignment tables, tuned against CoreSim.
# tt engine: which engine runs the expansion add for global tile t (0..31).
# dma engine: which engine issues the output DMA for tile t.
DEFAULT_TT = ["pool" if _t % 16 in (1, 3, 5, 7, 9, 11, 13) else "dve"
              for _t in range(32)]
DEFAULT_TT[31] = "pool"

# out-DMA engines: alternate SP/ACT, two mid-stream Pool DMAs
DEFAULT_DMA = ["sync" if _t % 2 == 0 else "scalar" for _t in range(32)]
for _t in (9, 19, 29):
    DEFAULT_DMA[_t] = "gpsimd"

_PROGRAM = None


def build_row(nc, pools, j, tt_map, dma_map, uit_eng):
    (qrtd, a2all, out) = pools["dram"]
    ident = pools["ident"]
    (qrtp, uitraw, p2sbp, uisbp, otp) = pools["sbuf"]
    (psui, ps2p, uptp) = pools["psum"]

    it = pools["loaded"][j]
    a2 = pools["a2t"][:, j * 128 : (j + 1) * 128]
    zq = pools["zq"][j]

    # --- Ui^T: uitr[c, f*16+i] = q[f*16+i] . R_h[i, c]  (K=64) ---
    uitr = uitraw.tile([I, L], BF16)
    qv = it[:, 0:1024].rearrange("d (f i) -> d i f", i=I)
    for hi in range(2):  # instrument half
        pst = psui.tile([I, 512], F32)
        for i8 in range(8):
            i = hi * 8 + i8
            nc.tensor.matmul(
                pst[:, i8 * 64 : (i8 + 1) * 64],
                it[:, 1024 + I * i : 1024 + I * (i + 1)],
                qv[:, i, :],
            )
        ov = uitr[:].rearrange("c (f i) -> c i f", i=I)[
            :, hi * 8 : (hi + 1) * 8, :
        ]
        iv = pst[:].rearrange("c (i8 f) -> c i8 f", f=64)
        eng = uit_eng if isinstance(uit_eng, str) else uit_eng[hi]
        copy = nc.scalar.copy if eng == "scalar" else nc.vector.tensor_copy
        if j == 0:
            # row 0: copy f<8 first so tile 0's transpose unblocks early
            copy(ov[:, :, 0:8], iv[:, :, 0:8])
            copy(ov[:, :, 8:64], iv[:, :, 8:64])
        else:
            copy(ov, iv)

    # --- psh for the first half's 4 tiles (PE work emitted before the
    # transposes so PE never stalls waiting for the uiT copies) ---
    ps2_halves = []
    ps2 = ps2p.tile([128, 256], F32, name="ps2")
    for tq in range(4):
        lt = tq
        for g in range(4):
            fp = lt * 4 + g
            nc.tensor.matmul(
                ps2[g * 32 : (g + 1) * 32, tq * 64 : (tq + 1) * 64],
                zq[:, fp * 32 : (fp + 1) * 32],
                a2[:, 64 - 2 * fp : 128 - 2 * fp],
                tile_position=(0, g * 32),
            )
    ps2_halves.append(ps2)

    # --- ui, l-partitioned: 8 transposes into one PSUM tile, one evac ---
    upt = uptp.tile([128, 128], BF16)
    for lt in range(8):
        nc.tensor.transpose(
            upt[:, lt * I : (lt + 1) * I],
            uitr[:, lt * 128 : (lt + 1) * 128],
            ident[:],
        )
    uisb = uisbp.tile([128, 128], BF16)
    nc.vector.tensor_copy(uisb[:], upt[:])

    for half in range(2):
        if half == 1:
            ps2 = ps2p.tile([128, 256], F32, name="ps2")
            for tq in range(4):
                lt = 4 + tq
                for g in range(4):
                    fp = lt * 4 + g
                    nc.tensor.matmul(
                        ps2[g * 32 : (g + 1) * 32, tq * 64 : (tq + 1) * 64],
                        zq[:, fp * 32 : (fp + 1) * 32],
                        a2[:, 64 - 2 * fp : 128 - 2 * fp],
                        tile_position=(0, g * 32),
                    )
        else:
            ps2 = ps2_halves[0]
        # p2sb[p, 128*tq + 2f + c2] = psh pairs, bf16
        p2sb = p2sbp.tile([128, 512], BF16)
        p2v = p2sb[:].rearrange("p (tq f c2) -> p tq f c2", tq=4, c2=2)
        p2src = (
            ps2[:].rearrange("p (tq f) -> p tq f", tq=4)
            .unsqueeze(3).broadcast_to([128, 4, 64, 2])
        )
        if j == 0 and half == 0:
            # row 0: evacuate tile 0's psh first so its tt unblocks early
            nc.vector.tensor_copy(p2v[:, 0:1], p2src[:, 0:1])
            nc.vector.tensor_copy(p2v[:, 1:4], p2src[:, 1:4])
        else:
            nc.vector.tensor_copy(p2v, p2src)

        for tq in range(4):
            lt = half * 4 + tq
            t = j * 8 + lt
            # ot[p, f*16 + 2*ch + c2] = psh2[p, 2f+c2] + ui[p, 2ch+c2]
            ot = otp.tile([128, L], BF16)
            ov = ot[:].rearrange("p (f ch c2) -> p f ch c2", ch=8, c2=2)
            in1 = (
                uisb[:, lt * I : (lt + 1) * I]
                .rearrange("p (ch c2) -> p ch c2", c2=2)
                .unsqueeze(1)
                .broadcast_to([128, 64, 8, 2])
            )
            in0 = (
                p2sb[:, tq * 128 : (tq + 1) * 128]
                .rearrange("p (f c2) -> p f c2", c2=2)
                .unsqueeze(2)
                .broadcast_to([128, 64, 8, 2])
            )
            if tt_map[t] == "pool":
                nc.gpsimd.tensor_tensor(ov, in0, in1, mybir.AluOpType.add)
            else:
                nc.vector.tensor_tensor(ov, in0, in1, mybir.AluOpType.add)

            dst = out[j].rearrange("(t p) m -> p t m", p=128)[:, lt, :]
            getattr(nc, dma_map[t]).dma_start(dst, ot[:])


def build_program(loop_iters: int | None = None, tt_map=None, dma_map=None,
                  uit_eng=("scalar", "scalar"), obufs: int = 10) -> bass.Bass:
    """loop_iters: device-side repeat loop for benchmarking only."""
    tt_map = tt_map or DEFAULT_TT
    dma_map = dma_map or DEFAULT_DMA
    nc = bacc.Bacc("TRN2", debug=False, num_devices=N_CORES)
    qrtd = nc.declare_dram_parameter(
        "qrt", [ROWS_PER_CORE, D, IT_COLS], BF16, isOutput=False
    )
    zqd = nc.declare_dram_parameter(
        "zqd", [ROWS_PER_CORE, 128, L], BF16, isOutput=False
    )
    a2all = nc.declare_dram_parameter(
        "a2all", [128, ROWS_PER_CORE * 128], BF16, isOutput=False
    )
    out = nc.declare_dram_parameter(
        "out", [ROWS_PER_CORE, L, L], BF16, isOutput=True
    )

    with TileContext(nc) as tc:
        with (
            tc.tile_pool(name="const", bufs=1) as constp,
            tc.tile_pool(name="qrtp", bufs=ROWS_PER_CORE) as qrtp,
            tc.tile_pool(name="uitraw", bufs=2) as uitraw,
            tc.tile_pool(name="p2sb", bufs=3) as p2sbp,
            tc.tile_pool(name="uisb", bufs=2) as uisbp,
            tc.tile_pool(name="otp", bufs=obufs) as otp,
            tc.tile_pool(name="psui", bufs=2, space="PSUM") as psui,
            tc.tile_pool(name="ps2", bufs=3, space="PSUM") as ps2p,
            tc.tile_pool(name="upt", bufs=2, space="PSUM") as uptp,
        ):
            ident = constp.tile([I, I], BF16)
            masks.make_identity(nc, ident[:])
            a2t = constp.tile([128, ROWS_PER_CORE * 128], BF16)
            zqs = [constp.tile([128, L], BF16, name=f"zqt{j}")
                   for j in range(ROWS_PER_CORE)]

            pools = {
                "dram": (qrtd, a2all, out),
                "sbuf": (qrtp, uitraw, p2sbp, uisbp, otp),
                "psum": (psui, ps2p, uptp),
                "ident": ident,
                "a2t": a2t,
                "zq": zqs,
            }

            def body(_iv=None):
                pools["loaded"] = []
                nc.scalar.dma_start(a2t[:], a2all[:])
                for j in range(ROWS_PER_CORE):
                    eng = nc.sync if j < 3 else nc.scalar
                    it = qrtp.tile([D, IT_COLS], BF16, name=f"it{j}")
                    # qrt first: the ui chain (row-critical) needs it before
                    # the psh matmuls need zq
                    eng.dma_start(it[:], qrtd[j])
                    eng.dma_start(zqs[j][:], zqd[j])
                    pools["loaded"].append(it)
                for j in range(ROWS_PER_CORE):
                    build_row(nc, pools, j, tt_map, dma_map, uit_eng)

            if loop_iters is None:
                body()
            else:
                with tc.For_i(0, loop_iters, 1) as _iv:
                    body(_iv)
    return nc


def make_in_maps(q, r_instrument, e_past):
    """Host-side sharding + table prep. Returns per-core input dicts."""
    q = np.asarray(q, dtype=np.float32)
    r_instrument = np.asarray(r_instrument, dtype=np.float32)
    e_past = np.asarray(e_past, dtype=np.float32)

    qT = q.reshape(32, L, D).transpose(0, 2, 1)  # (32, D, L)

    # zq[r, s*64+d, l] = qT[r, d, l] where (l//16) % 2 == s, else 0
    par = (np.arange(L) // I) % 2
    zq = np.zeros((32, 128, L), np.float32)
    for s in (0, 1):
        cols = par == s
        zq[:, s * 64 : (s + 1) * 64, cols] = qT[:, :, cols]

    # rt[h, d, 16i+c] = R[i, c, d, h]
    rt = r_instrument.transpose(3, 2, 0, 1).reshape(8, D, I * I)  # (8, D, 256)

    # a2[h, s*64+d, t] = a_h[(64 - t + s) % 65, d], t in [0, 128)
    a = np.concatenate(
        [e_past, np.full((1, D, 8), PAD_VAL, dtype=np.float32)], axis=0
    )  # (65, D, H)
    idx2 = (64 - np.arange(128)[None, :] + np.arange(2)[:, None]) % 65
    a2 = a[idx2]  # (2, 128, D, 8)
    a2 = a2.transpose(3, 0, 2, 1).reshape(8, 128, 128)  # (h, s*64+d, t)

    in_maps = []
    for k in range(N_CORES):
        rows = [ROWS_PER_CORE * k + j for j in range(ROWS_PER_CORE)]
        hs = [r % 8 for r in rows]
        # it[d] = [qT[d, :] | rt[d, :]]
        its = []
        for r, h in zip(rows, hs):
            its.append(
                np.concatenate([qT[r], rt[h]], axis=1)[None]
            )  # (1, 64, 1280)
        a2c = np.concatenate([a2[h] for h in hs], axis=1)  # (128, 512)
        in_maps.append(
            {
                "qrt": np.ascontiguousarray(
                    np.concatenate(its, axis=0).astype(NP_BF16)
                ),
                "a2all": np.ascontiguousarray(a2c.astype(NP_BF16)),
                "zqd": np.ascontiguousarray(zq[rows].astype(NP_BF16)),
            }
        )
    return in_maps


def _get_program() -> bass.Bass:
    global _PROGRAM
    if _PROGRAM is None:
        _PROGRAM = build_program()
        if not _PROGRAM.is_finalized():
            _PROGRAM.finalize()
    return _PROGRAM


def kernel(q, r_instrument, e_past, flipped_masks=None, **_unused):
    in_maps = make_in_maps(q, r_instrument, e_past)
    res = run_bass_kernel_spmd(_get_program(), in_maps, list(range(N_CORES))).results
    out = np.concatenate(
        [np.asarray(res[k]["out"], dtype=np.float32) for k in range(N_CORES)],
        axis=0,
    )
    return out.reshape(N_CORES * ROWS_PER_CORE, L, L)

